# revision 3
# baseline (speedup 1.0000x reference)
"""Trainium2 Bass kernel for nn_LinearFlowModel (dense_mlp).

Computes, for B=131072 cells and D=128 per-node models:
    out = einsum('bd,nod->bno', state, W) + b   -> delta = out[:,:,0], var = out[:,:,1]

which is a single matmul  state[B,128] @ Wmat[128,256] + bias  with
Wmat[d, o*128+n] = W[n,o,d] (o-major output columns so delta/var are the
two contiguous 128-column halves of the [B,256] result).

Sharding: pure data parallel over 8 NeuronCores — batch split into 8 shards
of 16384 rows; W/b replicated; no cross-device communication.

Per-core kernel (fp32 end to end):
  for each chunk of 512 batch rows (32 chunks):
    - one DMA loads [128p, 4, 128d] of state (256 KB)
    - PE transposes each [128,128] subtile into PSUM (state^T needed since
      the matmul contracts over d, which arrives in the free dimension)
    - ScalarE evacuates transposed pairs PSUM->SBUF
    - PE matmul: out[128b, 256m] = stateT.T @ Wmat  (fp32)
    - VectorE adds the (partition-broadcast) bias while evacuating PSUM->SBUF
    - one DMA stores [128p, 4, 256m] of the output (512 KB)
"""

import sys

if "/opt/trn_rl_repo" not in sys.path:
    sys.path.insert(0, "/opt/trn_rl_repo")

import numpy as np

B = 131072
D = 128
M = 256  # 2 heads * 128 nodes, o-major
NCORES = 8
BLOC = B // NCORES  # 16384 rows per core
SUB = 128  # rows per PE transpose/matmul tile
SUBS_PER_CHUNK = 4  # subtiles per DMA chunk
CHUNK = SUB * SUBS_PER_CHUNK  # 512
NCHUNK = BLOC // CHUNK  # 32

_prog = None  # cached (nc,) so repeated kernel() calls reuse the Bass module


def _build_program():
    import concourse.bacc as bacc
    import concourse.mybir as mybir
    from concourse import tile

    f32 = mybir.dt.float32

    nc = bacc.Bacc(
        "TRN2",
        target_bir_lowering=False,
        debug=False,
        num_devices=NCORES,
    )

    state_d = nc.dram_tensor("state", [BLOC, D], f32, kind="ExternalInput").ap()
    wmat_d = nc.dram_tensor("wmat", [D, M], f32, kind="ExternalInput").ap()
    bias2_d = nc.dram_tensor("bias2", [128, 2 * M], f32, kind="ExternalInput").ap()
    ident_d = nc.dram_tensor("ident", [128, 128], f32, kind="ExternalInput").ap()
    out_d = nc.dram_tensor("out", [BLOC, M], f32, kind="ExternalOutput").ap()

    # [p, a, d] view: batch row = a*128 + p
    state_v = state_d.rearrange("(a p) d -> p a d", p=128)
    out_v = out_d.rearrange("(a p) m -> p a m", p=128)

    with tile.TileContext(nc) as tc:
        with (
            tc.tile_pool(name="const", bufs=1) as cpool,
            tc.tile_pool(name="xin", bufs=4) as xpool,
            tc.tile_pool(name="xt", bufs=4) as xtpool,
            tc.tile_pool(name="yout", bufs=4) as ypool,
            tc.tile_pool(name="pst", bufs=4, space="PSUM") as pstpool,
            tc.tile_pool(name="psm", bufs=4, space="PSUM") as psmpool,
        ):
            wmat_sb = cpool.tile([D, M], f32)
            nc.sync.dma_start(wmat_sb[:], wmat_d[:])
            bias2_sb = cpool.tile([128, 2, M], f32)
            nc.sync.dma_start(bias2_sb[:], bias2_d.rearrange("p (j m) -> p j m", j=2))
            ident_sb = cpool.tile([128, 128], f32)
            nc.sync.dma_start(ident_sb[:], ident_d[:])

            for c in range(NCHUNK):
                x = xpool.tile([128, SUBS_PER_CHUNK, SUB], f32)
                nc.sync.dma_start(
                    x[:], state_v[:, c * SUBS_PER_CHUNK : (c + 1) * SUBS_PER_CHUNK, :]
                )
                y = ypool.tile([128, SUBS_PER_CHUNK, M], f32)
                for h in range(SUBS_PER_CHUNK // 2):
                    xt_ps = pstpool.tile([128, 2, SUB], f32)
                    nc.tensor.transpose(xt_ps[:, 0, :], x[:, 2 * h, :], ident_sb[:])
                    nc.tensor.transpose(xt_ps[:, 1, :], x[:, 2 * h + 1, :], ident_sb[:])
                    xt_sb = xtpool.tile([128, 2, SUB], f32)
                    nc.scalar.copy(xt_sb[:], xt_ps[:])
                    mm_ps = psmpool.tile([128, 2, M], f32)
                    nc.tensor.matmul(
                        mm_ps[:, 0, :], xt_sb[:, 0, :], wmat_sb[:], start=True, stop=True
                    )
                    nc.tensor.matmul(
                        mm_ps[:, 1, :], xt_sb[:, 1, :], wmat_sb[:], start=True, stop=True
                    )
                    nc.vector.tensor_add(
                        y[:, 2 * h : 2 * h + 2, :], mm_ps[:], bias2_sb[:]
                    )
                nc.sync.dma_start(
                    out_v[:, c * SUBS_PER_CHUNK : (c + 1) * SUBS_PER_CHUNK, :], y[:]
                )

    nc.compile()
    return nc


def _get_program():
    global _prog
    if _prog is None:
        _prog = _build_program()
    return _prog


def _prep_inputs(state, W, b):
    state = np.ascontiguousarray(np.asarray(state, dtype=np.float32))
    W = np.asarray(W, dtype=np.float32)
    b = np.asarray(b, dtype=np.float32)
    # Wmat[d, o*128+n] = W[n, o, d]
    wmat = np.ascontiguousarray(W.transpose(2, 1, 0).reshape(D, M))
    biasv = np.ascontiguousarray(b.transpose(1, 0).reshape(M))  # [o*128+n]
    bias2 = np.ascontiguousarray(
        np.broadcast_to(np.tile(biasv, 2)[None, :], (128, 2 * M))
    )
    ident = np.eye(128, dtype=np.float32)
    shards = state.reshape(NCORES, BLOC, D)
    in_maps = [
        {"state": shards[i], "wmat": wmat, "bias2": bias2, "ident": ident}
        for i in range(NCORES)
    ]
    return in_maps


def run_on_device(state, W, b, trace=False, **kw):
    """Run the Bass kernel on the 8 NeuronCores; returns (results, BassKernelResults)."""
    from concourse.bass_utils import run_bass_kernel_spmd

    nc = _get_program()
    in_maps = _prep_inputs(state, W, b)
    res = run_bass_kernel_spmd(nc, in_maps, list(range(NCORES)), trace=trace, **kw)
    full = np.concatenate([r["out"] for r in res.results], axis=0)  # [B, 256]
    return full, res


def kernel(state, W, b):
    full, _ = run_on_device(state, W, b, trace=False)
    delta = full[:, :D]
    var = full[:, D:]
    return delta, var


# revision 8
# speedup vs baseline: 1.0513x; 1.0513x over previous
"""Trainium2 Bass kernel for nn_LinearFlowModel (dense_mlp).

Computes, for B=131072 cells and D=128 per-node models:
    out = einsum('bd,nod->bno', state, W) + b   -> delta = out[:,:,0], var = out[:,:,1]

which is a single matmul  state[B,128] @ Wmat[128,256] + bias  with
Wmat[d, o*128+n] = W[n,o,d] (o-major output columns so delta/var are the
two contiguous 128-column halves of the [B,256] result).

Sharding: pure data parallel over 8 NeuronCores — batch split into 8 shards
of 16384 rows; W/b replicated; no cross-device communication.

Per-core kernel (fp32 end to end):
  for each chunk of 512 batch rows (32 chunks):
    - one DMA loads [128p, 4, 128d] of state (256 KB)
    - PE transposes each [128,128] subtile into PSUM (state^T needed since
      the matmul contracts over d, which arrives in the free dimension)
    - ScalarE evacuates transposed pairs PSUM->SBUF
    - PE matmul: out[128b, 256m] = stateT.T @ Wmat  (fp32)
    - VectorE adds the (partition-broadcast) bias while evacuating PSUM->SBUF
    - one DMA stores [128p, 4, 256m] of the output (512 KB)
"""

import sys

if "/opt/trn_rl_repo" not in sys.path:
    sys.path.insert(0, "/opt/trn_rl_repo")

import numpy as np

B = 131072
D = 128
M = 256  # 2 heads * 128 nodes, o-major
NCORES = 8
BLOC = B // NCORES  # 16384 rows per core
SUB = 128  # rows per PE transpose/matmul tile
SUBS_PER_CHUNK = 4  # subtiles per DMA chunk
CHUNK = SUB * SUBS_PER_CHUNK  # 512
NCHUNK = BLOC // CHUNK  # 32

_prog = None  # cached (nc,) so repeated kernel() calls reuse the Bass module


def _build_program():
    import os

    import concourse.bacc as bacc
    import concourse.mybir as mybir
    from concourse import tile

    f32 = mybir.dt.float32
    # float32r: single-pass fp32 matmul mode — 4x faster than the fp32
    # LOW_HIGH split at moving-dim >= 256. Set MM_F32R=0 to fall back.
    mm_dt = mybir.dt.float32r if os.environ.get("MM_F32R", "1") == "1" else f32

    nc = bacc.Bacc(
        "TRN2",
        target_bir_lowering=False,
        debug=False,
        num_devices=NCORES,
    )

    state_d = nc.dram_tensor("state", [BLOC, D], f32, kind="ExternalInput").ap()
    wmat_d = nc.dram_tensor("wmat", [D, M], mm_dt, kind="ExternalInput").ap()
    bias2_d = nc.dram_tensor("bias2", [128, 2 * M], f32, kind="ExternalInput").ap()
    ident_d = nc.dram_tensor("ident", [128, 128], f32, kind="ExternalInput").ap()
    out_d = nc.dram_tensor("out", [BLOC, M], f32, kind="ExternalOutput").ap()

    # [p, a, d] view: batch row = a*128 + p
    state_v = state_d.rearrange("(a p) d -> p a d", p=128)
    out_v = out_d.rearrange("(a p) m -> p a m", p=128)

    with tile.TileContext(nc) as tc:
        with (
            tc.tile_pool(name="const", bufs=1) as cpool,
            tc.tile_pool(name="xin", bufs=4) as xpool,
            tc.tile_pool(name="xt", bufs=4) as xtpool,
            tc.tile_pool(name="yout", bufs=4) as ypool,
            tc.tile_pool(name="pst", bufs=4, space="PSUM") as pstpool,
            tc.tile_pool(name="psm", bufs=4, space="PSUM") as psmpool,
        ):
            wmat_sb = cpool.tile([D, M], mm_dt)
            nc.sync.dma_start(wmat_sb[:], wmat_d[:])
            bias2_sb = cpool.tile([128, 2, M], f32)
            nc.sync.dma_start(bias2_sb[:], bias2_d.rearrange("p (j m) -> p j m", j=2))
            ident_sb = cpool.tile([128, 128], f32)
            nc.sync.dma_start(ident_sb[:], ident_d[:])

            for c in range(NCHUNK):
                x = xpool.tile([128, SUBS_PER_CHUNK, SUB], f32)
                nc.sync.dma_start(
                    x[:], state_v[:, c * SUBS_PER_CHUNK : (c + 1) * SUBS_PER_CHUNK, :]
                )
                y = ypool.tile([128, SUBS_PER_CHUNK, M], f32)
                for h in range(SUBS_PER_CHUNK // 2):
                    xt_ps = pstpool.tile([128, 2, SUB], f32)
                    nc.tensor.transpose(xt_ps[:, 0, :], x[:, 2 * h, :], ident_sb[:])
                    nc.tensor.transpose(xt_ps[:, 1, :], x[:, 2 * h + 1, :], ident_sb[:])
                    xt_sb = xtpool.tile([128, 2, SUB], mm_dt)
                    nc.scalar.copy(xt_sb[:], xt_ps[:])
                    mm_ps = psmpool.tile([128, 2, M], f32)
                    nc.tensor.matmul(
                        mm_ps[:, 0, :], xt_sb[:, 0, :], wmat_sb[:], start=True, stop=True
                    )
                    nc.tensor.matmul(
                        mm_ps[:, 1, :], xt_sb[:, 1, :], wmat_sb[:], start=True, stop=True
                    )
                    nc.vector.tensor_add(
                        y[:, 2 * h : 2 * h + 2, :], mm_ps[:], bias2_sb[:]
                    )
                nc.sync.dma_start(
                    out_v[:, c * SUBS_PER_CHUNK : (c + 1) * SUBS_PER_CHUNK, :], y[:]
                )

    nc.compile()
    return nc


def _get_program():
    global _prog
    if _prog is None:
        _prog = _build_program()
    return _prog


def _prep_inputs(state, W, b):
    state = np.ascontiguousarray(np.asarray(state, dtype=np.float32))
    W = np.asarray(W, dtype=np.float32)
    b = np.asarray(b, dtype=np.float32)
    # Wmat[d, o*128+n] = W[n, o, d]
    wmat = np.ascontiguousarray(W.transpose(2, 1, 0).reshape(D, M))
    biasv = np.ascontiguousarray(b.transpose(1, 0).reshape(M))  # [o*128+n]
    bias2 = np.ascontiguousarray(
        np.broadcast_to(np.tile(biasv, 2)[None, :], (128, 2 * M))
    )
    ident = np.eye(128, dtype=np.float32)
    shards = state.reshape(NCORES, BLOC, D)
    in_maps = [
        {"state": shards[i], "wmat": wmat, "bias2": bias2, "ident": ident}
        for i in range(NCORES)
    ]
    return in_maps


def run_on_device(state, W, b, trace=False, **kw):
    """Run the Bass kernel on the 8 NeuronCores; returns (results, BassKernelResults)."""
    from concourse.bass_utils import run_bass_kernel_spmd

    nc = _get_program()
    in_maps = _prep_inputs(state, W, b)
    res = run_bass_kernel_spmd(nc, in_maps, list(range(NCORES)), trace=trace, **kw)
    full = np.concatenate([r["out"] for r in res.results], axis=0)  # [B, 256]
    return full, res


def kernel(state, W, b):
    full, _ = run_on_device(state, W, b, trace=False)
    delta = full[:, :D]
    var = full[:, D:]
    return delta, var


# revision 9
# speedup vs baseline: 1.4170x; 1.3478x over previous
"""Trainium2 Bass kernel for nn_LinearFlowModel (dense_mlp).

Computes, for B=131072 cells and D=128 per-node models:
    out = einsum('bd,nod->bno', state, W) + b   -> delta = out[:,:,0], var = out[:,:,1]

which is a single matmul  state[B,128] @ Wmat[128,256] + bias  with
Wmat[d, o*128+n] = W[n,o,d] (o-major output columns so delta/var are the
two contiguous 128-column halves of the [B,256] result).

Sharding: pure data parallel over 8 NeuronCores — batch split into 8 shards
of 16384 rows; W/b replicated; no cross-device communication.

Per-core kernel. The matmul contracts over d, which arrives in the free
dimension, so state tiles are transposed on the PE array first. Batch rows
are assigned to partitions as b = 16*p + r (r = 0..15 within a 2048-row
chunk) which makes both the input DMA (1 MB) and output DMA (2 MB) a single
contiguous run per partition — minimal descriptor-generation time on the
HWDGE rings. Inputs issue on the sync ring, outputs on the scalar ring so
the two streams don't head-of-line block each other.

  per 2048-row chunk (8 chunks):
    - 1 in-DMA  [128p, 16r, 128d]            (sync ring)
    - 16 PE transposes -> stateT pairs in PSUM
    - 8 ScalarE copies evacuate stateT PSUM->SBUF (rounding to fp32r)
    - 16 PE matmuls  out[128b', 256m] = stateT.T @ Wmat   (fp32r, N=256)
    - 8 VectorE adds apply the bias while evacuating PSUM->SBUF
    - 1 out-DMA [128p, 16r, 256m]            (scalar ring)
"""

import sys

if "/opt/trn_rl_repo" not in sys.path:
    sys.path.insert(0, "/opt/trn_rl_repo")

import numpy as np

B = 131072
D = 128
M = 256  # 2 heads * 128 nodes, o-major
NCORES = 8
BLOC = B // NCORES  # 16384 rows per core
SUB = 128  # rows per PE transpose/matmul tile
RGRP = 16  # rows per partition per chunk
CHUNK = SUB * RGRP  # 2048
NCHUNK = BLOC // CHUNK  # 8

_prog = None  # cached so repeated kernel() calls reuse the compiled module


def _build_program():
    import os

    import concourse.bacc as bacc
    import concourse.mybir as mybir
    from concourse import tile

    f32 = mybir.dt.float32
    # float32r: single-pass fp32 matmul mode — 4x faster than the fp32
    # LOW_HIGH split at moving-dim >= 256, ~tf32 operand rounding.
    # MM_F32R=0 falls back to exact fp32.
    mm_dt = mybir.dt.float32r if os.environ.get("MM_F32R", "1") == "1" else f32

    nc = bacc.Bacc(
        "TRN2",
        target_bir_lowering=False,
        debug=False,
        num_devices=NCORES,
    )

    state_d = nc.dram_tensor("state", [BLOC, D], f32, kind="ExternalInput").ap()
    wmat_d = nc.dram_tensor("wmat", [D, M], mm_dt, kind="ExternalInput").ap()
    bias2_d = nc.dram_tensor("bias2", [128, 2 * M], f32, kind="ExternalInput").ap()
    ident_d = nc.dram_tensor("ident", [128, 128], f32, kind="ExternalInput").ap()
    out_d = nc.dram_tensor("out", [BLOC, M], f32, kind="ExternalOutput").ap()

    # batch row = c*CHUNK + RGRP*p + r  ->  [c, p, r, d] view
    state_v = state_d.rearrange("(c p r) d -> c p r d", p=128, r=RGRP)
    out_v = out_d.rearrange("(c p r) m -> c p r m", p=128, r=RGRP)

    with tile.TileContext(nc) as tc:
        with (
            tc.tile_pool(name="const", bufs=1) as cpool,
            tc.tile_pool(name="xin", bufs=3) as xpool,
            tc.tile_pool(name="xt", bufs=6) as xtpool,
            tc.tile_pool(name="yout", bufs=3) as ypool,
            tc.tile_pool(name="pst", bufs=3, space="PSUM") as pstpool,
            tc.tile_pool(name="psm", bufs=4, space="PSUM") as psmpool,
        ):
            wmat_sb = cpool.tile([D, M], mm_dt)
            nc.sync.dma_start(wmat_sb[:], wmat_d[:])
            bias2_sb = cpool.tile([128, 2, M], f32)
            nc.sync.dma_start(bias2_sb[:], bias2_d.rearrange("p (j m) -> p j m", j=2))
            ident_sb = cpool.tile([128, 128], f32)
            nc.sync.dma_start(ident_sb[:], ident_d[:])

            for c in range(NCHUNK):
                x = xpool.tile([128, RGRP, SUB], f32)
                nc.sync.dma_start(x[:], state_v[c])
                y = ypool.tile([128, RGRP, M], f32)
                for h in range(RGRP // 2):
                    xt_ps = pstpool.tile([128, 2, SUB], f32)
                    nc.tensor.transpose(xt_ps[:, 0, :], x[:, 2 * h, :], ident_sb[:])
                    nc.tensor.transpose(xt_ps[:, 1, :], x[:, 2 * h + 1, :], ident_sb[:])
                    xt_sb = xtpool.tile([128, 2, SUB], mm_dt)
                    nc.scalar.copy(xt_sb[:], xt_ps[:])
                    mm_ps = psmpool.tile([128, 2, M], f32)
                    nc.tensor.matmul(
                        mm_ps[:, 0, :], xt_sb[:, 0, :], wmat_sb[:], start=True, stop=True
                    )
                    nc.tensor.matmul(
                        mm_ps[:, 1, :], xt_sb[:, 1, :], wmat_sb[:], start=True, stop=True
                    )
                    nc.vector.tensor_add(
                        y[:, 2 * h : 2 * h + 2, :], mm_ps[:], bias2_sb[:]
                    )
                nc.scalar.dma_start(out_v[c], y[:])

    nc.compile()
    return nc


def _get_program():
    global _prog
    if _prog is None:
        _prog = _build_program()
    return _prog


def _prep_inputs(state, W, b):
    state = np.ascontiguousarray(np.asarray(state, dtype=np.float32))
    W = np.asarray(W, dtype=np.float32)
    b = np.asarray(b, dtype=np.float32)
    # Wmat[d, o*128+n] = W[n, o, d]
    wmat = np.ascontiguousarray(W.transpose(2, 1, 0).reshape(D, M))
    biasv = np.ascontiguousarray(b.transpose(1, 0).reshape(M))  # [o*128+n]
    bias2 = np.ascontiguousarray(
        np.broadcast_to(np.tile(biasv, 2)[None, :], (128, 2 * M))
    )
    ident = np.eye(128, dtype=np.float32)
    shards = state.reshape(NCORES, BLOC, D)
    in_maps = [
        {"state": shards[i], "wmat": wmat, "bias2": bias2, "ident": ident}
        for i in range(NCORES)
    ]
    return in_maps


def run_on_device(state, W, b, trace=False, **kw):
    """Run the Bass kernel on the 8 NeuronCores; returns (full_out, BassKernelResults)."""
    from concourse.bass_utils import run_bass_kernel_spmd

    nc = _get_program()
    in_maps = _prep_inputs(state, W, b)
    res = run_bass_kernel_spmd(nc, in_maps, list(range(NCORES)), trace=trace, **kw)
    full = np.concatenate([r["out"] for r in res.results], axis=0)  # [B, 256]
    return full, res


def kernel(state, W, b):
    full, _ = run_on_device(state, W, b, trace=False)
    delta = full[:, :D]
    var = full[:, D:]
    return delta, var
